# revision 38
# baseline (speedup 1.0000x reference)
"""AttentionWithSelfAblation TRN2 kernel.

Reference computation (B=4, S=2048, H=1024, nh=16, hd=64, window=256):
    q = x @ Wq.T ; k = x_clean @ Wk.T ; v = x_clean @ Wv.T   (per-head split)
    scores = q @ k.T  (NO 1/sqrt(hd) scaling)
    local causal mask: key j visible to query i iff i-255 <= j <= i
    attn = softmax(scores) ; ctx = attn @ v  (merge heads)
    out = (ctx * ablation_mask) @ Wo.T + bo
Sharding: pure data/sequence parallel over 8 cores: core c = (batch c//2,
sequence half c%2 of 1024 queries). Keys/values need a 256-halo to the left;
the first half uses zero-padding + masks instead. No collectives.

Dtypes: fp16 for x/xc/weights/qT/kT/ctx (scores accumulate fp32 in PSUM),
bf16 for v/exp (exp needs bf16 range: raw scores reach ~75; exp(s-20) bias
cancels in the softmax normalization). Measured end-to-end rel err ~3e-3.

Per-core device pipeline (all feature-major "T" layouts):
  warmup  : junk matmuls from a memset scratch keep the PE HAM busy while
            the first input DMAs land (PE otherwise idles ~15us and starts
            at half clock).
  phase Q : xT chunks streamed -> qT[o,s]; Wq arrives in 256KB quarters so
            the first matmul only waits for a quarter, not a 1MB half.
  phase KV: xcT chunks streamed -> kT[o,s] + v[s,o] (o augmented with a
            ones column per head: ctx matmul also yields the denominator)
  phase A : per (qpair of 256 queries, head pair): raw scoresT[sk,sq] by
            interleaved row-disjoint 64-row qk MMs (PE runs pairs
            concurrently); exp(s-20) on ACT -> bf16; multiplicative {0,1}
            band mask on DVE (no PE mask injection); ctx MMs split into
            64-row halves into two PSUM banks (A/B) interleaved for PE
            concurrency; den = A[64]+B[64] -> ones-MM broadcast to 64
            partitions -> DVE reciprocal; drain = (A+B) then *recip then
            *ablation -> fp16 ctx; out-proj (PE, fp16) + bias (ACT).
Host does all layout transposes (free) and unshards by concatenation.
"""

import numpy as np
import ml_dtypes

from concourse import bacc
import concourse.tile as tile
import concourse.mybir as mybir
from concourse.bass_utils import run_bass_kernel_spmd

B, S, H = 4, 2048, 1024
NH, HD = 16, 64
W = 256  # window
SL = 1024  # per-core sequence chunk
SKL = SL + W  # keys incl halo
NQP = SL // 256  # qpairs of 256 queries
NKT = 4  # k-tiles of 128 per qpair
NC = 8  # cores

F32 = mybir.dt.float32
F32R = mybir.dt.float32r
F16 = mybir.dt.float16
BF16 = mybir.dt.bfloat16
EXP = mybir.ActivationFunctionType.Exp
IDENT = mybir.ActivationFunctionType.Identity
MULT = mybir.AluOpType.mult

EXP_BIAS = -20.0  # exp(s + EXP_BIAS): cancels in softmax, avoids overflow
N_JUNK = 88  # HAM warm-up matmuls while input DMAs are in flight

_compiled = None


def _build():
    nc = bacc.Bacc("TRN2", target_bir_lowering=False, debug=False)

    xT = nc.dram_tensor("xT", [H, SL], F16, kind="ExternalInput")
    xcT = nc.dram_tensor("xcT", [H, SKL], F16, kind="ExternalInput")
    ablT = nc.dram_tensor("ablT", [H, SL], F16, kind="ExternalInput")
    WqT = nc.dram_tensor("WqT", [H, H], F16, kind="ExternalInput")
    WkT = nc.dram_tensor("WkT", [H, H], F16, kind="ExternalInput")
    WvT = nc.dram_tensor("WvT", [H, H], F16, kind="ExternalInput")
    WoT = nc.dram_tensor("WoT", [H, H], F16, kind="ExternalInput")
    bo = nc.dram_tensor("bo", [128, 8], F32, kind="ExternalInput")
    # masks[set, qsub, kt, sk, sq] multiplicative {1,0}: set 1 = qp==0
    masks = nc.dram_tensor("masks", [2, 2, 3, 128, 128], BF16, kind="ExternalInput")
    outT = nc.dram_tensor("outT", [H, SL], F32, kind="ExternalOutput")

    xT_d = xT.rearrange("(c p) s -> p c s", p=128)
    xcT_d = xcT.rearrange("(c p) s -> p c s", p=128)
    ablT_d = ablT.rearrange("(t p) s -> p t s", p=128)
    outT_d = outT.rearrange("(t p) s -> p t s", p=128)

    with tile.TileContext(nc) as tc:
        with (
            tc.tile_pool(name="consts", bufs=1) as consts,
            tc.tile_pool(name="big", bufs=1) as big,
            tc.tile_pool(name="wpool", bufs=6) as wpool,
            tc.tile_pool(name="xs", bufs=2) as xspool,
            tc.tile_pool(name="xcs", bufs=3) as xcspool,
            tc.tile_pool(name="outp", bufs=3) as outpool,
            tc.tile_pool(name="ps512", bufs=2, space="PSUM") as ps512,
        ):

            qT_sb = big.tile([128, 8, SL], F16)
            kT_sb = big.tile([128, 8, SKL], F16)
            v_sb = big.tile([128, 10, 16 * 65], BF16)

            # ---- junk warm-up: PE runs garbage MMs off the ones scratch
            # while the first x/Wq DMAs land; keeps HAM busy so real MMs
            # start immediately and at full (2.4GHz) clock.
            ones_scratch = consts.tile([128, 160], BF16)
            nc.vector.memset(ones_scratch[:], 1.0)
            junk_ps = ps512.tile([128, 512], F32, name="junk_ps", tag="ps512")
            for _ in range(N_JUNK):
                nc.tensor.matmul(
                    junk_ps[:, :160], ones_scratch[:, 0:128], ones_scratch[:],
                    start=True, stop=True,
                )

            # ---- input DMA issue order (per-queue program order = issue
            # order). sync ring: x first (phase Q rhs; first 512-chunk is
            # issued as two 256-col DMAs so the first MMs gate on 512KB),
            # then xc. scalar ring: wq hf0 in 256KB oi-quarters (first MM
            # gates on one quarter), bo, wq hf1, wk/wv/wo halves, masks
            # last (not needed until phase A).
            # wq hf0 quarters interleave with the x stream on the sync
            # ring (odd quarters) and lead the scalar ring (even quarters):
            # one ring alone delivers 256KB quarters 1.3us apart while the
            # PE consumes each in ~0.45us.
            wq0_sb = wpool.tile([128, 8, 512], F16, name="w_WqT_0", tag="w")
            wq_src = WqT.rearrange("(c p) o -> p c o", p=128)

            def wq0_quarter(ring, oi):
                ring.dma_start(
                    wq0_sb[:, :, oi * 128 : oi * 128 + 128],
                    wq_src[:, :, oi * 128 : oi * 128 + 128],
                )

            wq0_quarter(nc.scalar, 0)
            x_ss = []
            x_subs = [(0, 128), (128, 128), (256, 256)]
            for ci in range(SL // 512):
                x_s = xspool.tile([128, 8, 512], F16, name=f"x_{ci}", tag="xs")
                if ci == 0:
                    for n, (s0, sn) in enumerate(x_subs):
                        ssl = slice(s0, s0 + sn)
                        nc.sync.dma_start(x_s[:, :, ssl], xT_d[:, :, ssl])
                        if n < 2:
                            wq0_quarter(nc.sync, 1 + 2 * n)
                else:
                    for sub in range(2):
                        ssl = slice(sub * 256, sub * 256 + 256)
                        nc.sync.dma_start(
                            x_s[:, :, ssl],
                            xT_d[:, :, ci * 512 + sub * 256 : ci * 512 + sub * 256 + 256],
                        )
                x_ss.append(x_s)
            kv_chunks = [(0, 512), (512, 512), (1024, 256)]
            xc_ss = []
            for ci, (s0c, snc) in enumerate(kv_chunks):
                xc_s = xcspool.tile(
                    [128, 8, 512], F16, name=f"xc_{ci}", tag="xcs"
                )
                nc.sync.dma_start(
                    xc_s[:, :, :snc], xcT_d[:, :, s0c : s0c + snc]
                )
                xc_ss.append(xc_s)

            def load_weight_half(dram, hf, split=False):
                """o-columns [hf*512, (hf+1)*512) of a transposed weight on
                the scalar (qScalarDynamicHW) ring so weight prefetch never
                blocks the x/xc stream on sync. split=True issues the four
                oi-quarters separately (the consumer MMs gate per-region)."""
                w_sb = wpool.tile(
                    [128, 8, 512], F16, name=f"w_{dram.name}_{hf}", tag="w"
                )
                src = dram.rearrange("(c p) o -> p c o", p=128)
                if split:
                    for oi in range(4):
                        osl = slice(oi * 128, oi * 128 + 128)
                        nc.scalar.dma_start(
                            w_sb[:, :, osl],
                            src[:, :, hf * 512 + oi * 128 : hf * 512 + oi * 128 + 128],
                        )
                else:
                    nc.scalar.dma_start(
                        w_sb[:], src[:, :, hf * 512 : (hf + 1) * 512]
                    )
                return w_sb

            wq0_quarter(nc.scalar, 2)
            wq_hs = [wq0_sb]
            # bo pre-shuffled on host to [128, 8]: contiguous per-partition
            # rows (the naive "(t p) -> p t" pattern explodes into 4-byte
            # DMA descriptors costing ~7.6us of issue time)
            bo_sb = consts.tile([128, 8], F32)
            nc.scalar.dma_start(bo_sb[:], bo[:, :])
            wq_hs.append(load_weight_half(WqT, 1, split=True))
            wk_hs = [load_weight_half(WkT, hf) for hf in range(2)]
            wv_hs = [load_weight_half(WvT, hf) for hf in range(2)]
            wo_hs = [load_weight_half(WoT, hf) for hf in range(2)]
            mask_sb = consts.tile([128, 2, 2, 3, 128], BF16)
            nc.scalar.dma_start(
                mask_sb[:], masks.rearrange("s u t k q -> k s u t q")
            )

            # ones columns of the augmented v (slot 64 of each head's 65):
            # strided-copy from the ones scratch into place
            v_aug = v_sb[:].rearrange("p t (h e) -> p t h e", e=65)
            nc.vector.tensor_copy(
                v_aug[:, :, :, 64],
                ones_scratch[:].rearrange("p (t h) -> p t h", t=10),
            )
            # per-partition bias column for exp(s + EXP_BIAS)
            ebias = consts.tile([128, 1], F32)
            nc.vector.memset(ebias[:], EXP_BIAS)
            # pmask routes even-head denominators to partitions 0:64, odd
            # to 64:128 -- built with memsets (DMA of [1,2,128] costs ~7us
            # of descriptor issue time)
            pmask = consts.tile([1, 2, 128], BF16)
            nc.vector.memset(pmask[:], 0.0)
            nc.vector.memset(pmask[:, 0, 0:64], 1.0)
            nc.vector.memset(pmask[:, 1, 64:128], 1.0)

            # ---- phase Q ----
            # sub-chunks: (x tile idx, col offset within tile, width)
            q_chunks = [
                (0, 0, 128), (0, 128, 128), (0, 256, 256),
                (1, 0, 256), (1, 256, 256),
            ]
            for hf in range(2):
                wq_sb = wq_hs[hf]
                for ti, c0, snc in q_chunks:
                    x_s = x_ss[ti]
                    s0c = ti * 512 + c0
                    for oi in range(4):
                        ot = hf * 4 + oi
                        ps = ps512.tile([128, snc], F32, tag="ps512")
                        for c in range(8):
                            nc.tensor.matmul(
                                ps[:],
                                wq_sb[:, c, oi * 128 : (oi + 1) * 128],
                                x_s[:, c, c0 : c0 + snc],
                                start=(c == 0),
                                stop=(c == 7),
                            )
                        nc.vector.tensor_copy(
                            qT_sb[:, ot, s0c : s0c + snc], ps[:]
                        )

            # ---- phase KV: kT[o, s] + v[s, o] (o augmented per head) ----
            def emit_kproj(hf, ci, oi, s0=None, sn=None):
                s0c, snc = kv_chunks[ci]
                if s0 is None:
                    s0, sn = 0, snc
                xc_s = xc_ss[ci]
                wk_sb = wk_hs[hf]
                ot = hf * 4 + oi
                ps = ps512.tile([128, 512], F32, tag="ps512")
                for c in range(8):
                    nc.tensor.matmul(
                        ps[:, :sn],
                        wk_sb[:, c, oi * 128 : (oi + 1) * 128],
                        xc_s[:, c, s0 : s0 + sn],
                        start=(c == 0),
                        stop=(c == 7),
                    )
                nc.vector.tensor_copy(
                    kT_sb[:, ot, s0c + s0 : s0c + s0 + sn], ps[:, :sn]
                )

            def emit_vproj(hf, ci, sti, half=None):
                s0c, snc = kv_chunks[ci]
                xc_s = xc_ss[ci]
                wv_sb = wv_hs[hf]
                st = s0c // 128 + sti
                o0, on = (0, 512) if half is None else (half * 256, 256)
                ps = ps512.tile([128, 512], F32, tag="ps512")
                for c in range(8):
                    nc.tensor.matmul(
                        ps[:, :on],
                        xc_s[:, c, sti * 128 : (sti + 1) * 128],
                        wv_sb[:, c, o0 : o0 + on],
                        start=(c == 0),
                        stop=(c == 7),
                    )
                nc.scalar.copy(
                    v_aug[:, st, hf * 8 + o0 // 64 : hf * 8 + (o0 + on) // 64, 0:64],
                    ps[:, :on].rearrange("p (h e) -> p h e", e=64),
                )

            # hf0 chunks 0/1 inline; everything else (hf0 halo, the whole
            # hf1 K/V projection, hf1 halo) becomes PE filler inside phase
            # A's first pass (head pairs 0-3 only need hf0), giving the PE
            # a steady supply of dependency-free work while the exp chain
            # runs. Units carry a PE-cost (in N-columns) for budgeted pops.
            for ci in range(2):
                for oi in range(4):
                    emit_kproj(0, ci, oi)
                for sti in range(kv_chunks[ci][1] // 128):
                    emit_vproj(0, ci, sti)
            # units kept at <= 2048 columns (~0.85us) so the per-iteration
            # credit budget can match filler supply to demand smoothly
            # (4096-col units beat against the budget and left ~0.8us PE
            # holes every other iteration)
            kv_fillers = []
            for oi in range(4):
                for h in range(2):
                    kv_fillers.append(
                        (1024, lambda oi=oi, h=h: emit_kproj(0, 2, oi, h * 128, 128))
                    )
            for sti in range(2):
                for half in range(2):
                    kv_fillers.append(
                        (2048, lambda sti=sti, half=half: emit_vproj(0, 2, sti, half))
                    )
            for ci in range(2):
                for oi in range(4):
                    for h in range(2):
                        kv_fillers.append(
                            (2048, lambda ci=ci, oi=oi, h=h: emit_kproj(1, ci, oi, h * 256, 256))
                        )
                for sti in range(kv_chunks[ci][1] // 128):
                    for half in range(2):
                        kv_fillers.append(
                            (2048, lambda ci=ci, sti=sti, half=half: emit_vproj(1, ci, sti, half))
                        )
            for oi in range(4):
                for h in range(2):
                    kv_fillers.append(
                        (1024, lambda oi=oi, h=h: emit_kproj(1, 2, oi, h * 128, 128))
                    )
            for sti in range(2):
                for half in range(2):
                    kv_fillers.append(
                        (2048, lambda sti=sti, half=half: emit_vproj(1, 2, sti, half))
                    )

            # ---- phase A: attention + out-projection per qpair ----
            # q-subtiles of 128 queries x 3 k-tiles (the 256-query x 4-kt
            # tiling computes 512 key-dots per query; 128x3 computes only
            # 384): 25% less score/exp/mask/ctx work. The middle k-tile is
            # fully in-band -> no mask needed at all.
            with (
                tc.tile_pool(name="expr", bufs=3) as exprpool,
                tc.tile_pool(name="recip", bufs=3) as recippool,
                tc.tile_pool(name="abl", bufs=3) as ablpool,
                tc.tile_pool(name="ctxs", bufs=2) as ctxpool,
                tc.tile_pool(name="ps_sc", bufs=5, space="PSUM") as ps_sc,
                tc.tile_pool(name="ps_ctx", bufs=1, space="PSUM") as ps_ctx,
            ):
                def emit_outproj(ctx_tile, qg, ot, half):
                    """One out-projection half-group (N=256). Interleaved
                    between head iterations of the NEXT query group as
                    dependency-free PE filler: absorbs exp-latency stalls
                    and keeps the PE clock (HAM) warm through phase A."""
                    wo_sb = wo_hs[ot // 4]
                    oi = ot % 4
                    osl = slice(half * 256, half * 256 + 256)
                    ps = ps512.tile(
                        [128, 256], F32, name=f"op_{qg}_{ot}_{half}", tag="ps512"
                    )
                    for c in range(8):
                        nc.tensor.matmul(
                            ps[:],
                            wo_sb[:, c, oi * 128 : (oi + 1) * 128],
                            ctx_tile[:, c, osl],
                            start=(c == 0),
                            stop=(c == 7),
                        )
                    o_sb = outpool.tile(
                        [128, 256], F32, name=f"out_{qg}_{ot}_{half}", tag="outp"
                    )
                    nc.scalar.activation(
                        o_sb[:], ps[:], IDENT, bias=bo_sb[:, ot : ot + 1]
                    )
                    nc.sync.dma_start(
                        outT_d[:, ot, qg * 512 + half * 256 : qg * 512 + half * 256 + 256],
                        o_sb[:],
                    )

                # software pipeline over 32 (qp, t) iterations in TWO
                # passes: pass 1 = head pairs 0-3 (needs only hf0 k/v; the
                # hf1 K/V projection is its PE filler), pass 2 = head pairs
                # 4-7 (completed qpairs' out-projections are its filler).
                # This matches filler supply to per-iteration demand so the
                # PE never starves (starvation makes HAM drop to 1.2GHz).
                iters = [
                    (qp, t)
                    for p in range(2)
                    for qp in range(NQP)
                    for t in range(p * 4, p * 4 + 4)
                ]
                fillers = list(kv_fillers)
                fill_credit = [0]
                ctx_tiles = {}
                abl_tiles = {}
                mask_flat = mask_sb[:].rearrange("p s u t q -> p s (u t q)")

                def live_kts(qp, qsub):
                    # kt1 first: it is fully in-band (no mask) so its ctx
                    # MM only waits on exp, not the mask chain. (qp0's
                    # leading tiles must still be computed: the halo is
                    # real data on second-half cores; the per-half mask
                    # values handle the zero-padded case.)
                    return [1, 0, 2]

                def get_ctx(qg):
                    if qg not in ctx_tiles:
                        ctx_tiles[qg] = ctxpool.tile(
                            [128, 8, 512], F16, name=f"ctx_{qg}", tag="ctx"
                        )
                    return ctx_tiles[qg]

                def emit_scores(i):
                    qp, t = iters[i]
                    akey = (qp, t // 4)
                    if akey not in abl_tiles:
                        abl_q = ablpool.tile(
                            [128, 8, 256], F16, name=f"abl_{akey}", tag="abl"
                        )
                        nc.sync.dma_start(
                            abl_q[:], ablT_d[:, :, qp * 256 : qp * 256 + 256]
                        )
                        abl_tiles[akey] = abl_q
                    pss = [
                        [
                            ps_sc.tile(
                                [128, 4, 128], F32,
                                name=f"sc_{qp}_{2 * t + par}_{u}", tag="sc",
                            )
                            for u in range(2)
                        ]
                        for par in range(2)
                    ]
                    # NOTE: 4 allocations rotating over 5 buffers is
                    # deliberate: scores(i+1)'s first tile reuses a buffer
                    # freed during iteration i-1 (zero wait), later tiles
                    # chase iteration i's exps one step behind.
                    for par in range(2):
                        hsl = slice(par * 64, par * 64 + 64)
                        for qsub in range(2):
                            q0 = qp * 256 + qsub * 128
                            for kt in range(3):
                                nc.tensor.matmul(
                                    pss[par][qsub][:, kt, :],
                                    kT_sb[hsl, t, q0 + kt * 128 : q0 + kt * 128 + 128],
                                    qT_sb[hsl, t, q0 : q0 + 128],
                                    start=(kt == 0),
                                    stop=True,
                                    skip_group_check=True,
                                )
                    return pss

                def emit_expmask(i, pss):
                    """ACT exp + DVE multiplicative {0,1} band mask. The
                    middle k-tile is fully in-band: only kt0/kt2 are
                    masked (qp0 edge tiles mask all three)."""
                    qp, t = iters[i]
                    ms = 1 if qp == 0 else 0
                    exprs = []
                    for par in range(2):
                        h = 2 * t + par
                        expr_sb = exprpool.tile(
                            [128, 2, 3, 128], BF16,
                            name=f"er_{qp}_{h}", tag="expr",
                        )
                        er_flat = expr_sb[:].rearrange("p u t q -> p (u t q)")
                        for qsub in range(2):
                            nc.scalar.activation(
                                expr_sb[:, qsub],
                                pss[par][qsub][:, 0:3, :],
                                EXP,
                                bias=ebias[:],
                            )
                            f0 = qsub * 384
                            if ms:
                                slc = [slice(f0, f0 + 384)]
                            else:
                                slc = [
                                    slice(f0, f0 + 128),
                                    slice(f0 + 256, f0 + 384),
                                ]
                            for fsl in slc:
                                nc.vector.tensor_mul(
                                    er_flat[:, fsl],
                                    er_flat[:, fsl],
                                    mask_flat[:, ms, fsl],
                                )
                        exprs.append(expr_sb)
                    return exprs

                def emit_tail(i, exprs):
                    qp, t = iters[i]
                    qg, qph = qp // 2, qp % 2
                    qsl = slice(qph * 256, qph * 256 + 256)
                    ctx_sb = get_ctx(qg)
                    abl_q = abl_tiles[(qp, t // 4)]
                    psc = ps_ctx.tile(
                        [65, 2, 256], F32, name=f"ctxp_{qp}_{t}", tag="ctxp"
                    )
                    for par in range(2):
                        h = 2 * t + par
                        for qsub in range(2):
                            kts = live_kts(qp, qsub)
                            for n, kt in enumerate(kts):
                                nc.tensor.matmul(
                                    psc[:, par, qsub * 128 : qsub * 128 + 128],
                                    v_sb[
                                        :, qp * 2 + qsub + kt,
                                        h * 65 : h * 65 + 65,
                                    ],
                                    exprs[par][:, qsub, kt, :],
                                    start=(n == 0),
                                    stop=(n == len(kts) - 1),
                                )
                    # denominators (row 64) -> bf16 -> K=1 ones-MM
                    # broadcast to all 128 partitions -> wide reciprocal
                    rec = recippool.tile(
                        [1, 2, 256], BF16, name=f"rec_{qp}_{t}", tag="rec"
                    )
                    nc.vector.tensor_copy(rec[:], psc[64:65, :, :])
                    psb = ps512.tile(
                        [128, 256], F32, name=f"psb_{qp}_{t}", tag="ps512"
                    )
                    for par in range(2):
                        nc.tensor.matmul(
                            psb[:],
                            pmask[:, par, :],
                            rec[:, par, :],
                            start=(par == 0),
                            stop=(par == 1),
                        )
                    # drain pair to f32 scratch FIRST (frees the single
                    # psc buffer for the next iteration's ctx before the
                    # reciprocal chain), then compute the reciprocal.
                    # even head -> parts 0:64, odd -> 64:128
                    cs32 = recippool.tile(
                        [128, 256], F32, name=f"cs_{qp}_{t}", tag="cs"
                    )
                    nc.vector.tensor_copy(cs32[0:64, :], psc[0:64, 0, :])
                    nc.vector.tensor_copy(cs32[64:128, :], psc[0:64, 1, :])
                    rb = recippool.tile(
                        [128, 256], F32, name=f"rb_{qp}_{t}", tag="rb"
                    )
                    nc.vector.reciprocal_approx_fast(rb[:], psb[:])
                    # normalize on the write into fp16, then ablate —
                    # on GpSimd (all-SBUF operands, engine otherwise idle)
                    nc.gpsimd.tensor_mul(ctx_sb[:, t, qsl], cs32[:], rb[:])
                    nc.gpsimd.tensor_mul(
                        ctx_sb[:, t, qsl], ctx_sb[:, t, qsl], abl_q[:, t, :]
                    )
                    if t == NH // 2 - 1:
                        # this qpair's ctx is complete: its out-projection
                        # becomes PE filler for the following iterations
                        fillers.extend(
                            (
                                2048,
                                lambda c=ctx_sb, g=qg, o=ot, h=qph:
                                    emit_outproj(c, g, o, h),
                            )
                            for ot in range(8)
                        )

                def pop_fillers(budget_cols):
                    # pop dependency-free PE work worth ~budget columns
                    fill_credit[0] += budget_cols
                    while fillers and fill_credit[0] >= fillers[0][0]:
                        cost, fn = fillers.pop(0)
                        fill_credit[0] -= cost
                        fn()

                # per-iteration emission order (PE is in-order, so this IS
                # the PE schedule): exp/mask(i) first on ACT/DVE, then
                # budgeted fillers, then scores(i+1) (its PSUM-WAR stalls
                # resolve while fillers run), then ctx(i) + denominator.
                pss_cur = emit_scores(0)
                for i in range(len(iters)):
                    exprs = emit_expmask(i, pss_cur)
                    pop_fillers(5120 if i < 16 else 4608)
                    if i + 1 < len(iters):
                        pss_cur = emit_scores(i + 1)
                    emit_tail(i, exprs)
                for _, f in fillers:
                    f()
    nc.compile()
    return nc


def kernel(x, x_clean, ablation_mask, Wq, Wk, Wv, Wo, bo):
    global _compiled
    x = np.asarray(x, np.float16)
    x_clean = np.asarray(x_clean, np.float16)
    ablation_mask = np.asarray(ablation_mask, np.float16)
    WqT = np.ascontiguousarray(np.asarray(Wq, np.float16).T)
    WkT = np.ascontiguousarray(np.asarray(Wk, np.float16).T)
    WvT = np.ascontiguousarray(np.asarray(Wv, np.float16).T)
    WoT = np.ascontiguousarray(np.asarray(Wo, np.float16).T)
    bo2 = np.ascontiguousarray(np.asarray(bo, np.float32).reshape(8, 128).T)

    # masks: include iff 1 <= kt*128 + r - a <= 256 (q-subtiles of 128,
    # 3 k-tiles each); set 1 additionally excludes the zero-padded halo
    # keys (first sequence half, qp==0 only)
    r = np.arange(128)[:, None]
    a = np.arange(128)[None, :]
    masks_by_half = []
    for half in range(2):
        m = np.empty((2, 2, 3, 128, 128), np.float32)
        for u in range(2):
            for kt in range(3):
                d = kt * 128 + r - a
                inc = (d >= 1) & (d <= 256)
                m[0, u, kt] = np.where(inc, 1.0, 0.0)
                inc_edge = (
                    inc & ((u * 128 + kt * 128 + r) >= 256)
                    if half == 0
                    else inc
                )
                m[1, u, kt] = np.where(inc_edge, 1.0, 0.0)
        masks_by_half.append(m.astype(ml_dtypes.bfloat16))

    in_maps = []
    for c in range(NC):
        b, half = c // 2, c % 2
        s0 = half * SL
        xTc = np.ascontiguousarray(x[b, s0 : s0 + SL].T)
        xc = np.zeros((SKL, H), np.float16)
        lo = max(0, s0 - W)
        xc[W - (s0 - lo) :] = x_clean[b, lo : s0 + SL]
        xcTc = np.ascontiguousarray(xc.T)
        ablTc = np.ascontiguousarray(ablation_mask[b, s0 : s0 + SL].T)
        in_maps.append(
            {
                "xT": xTc,
                "xcT": xcTc,
                "ablT": ablTc,
                "WqT": WqT,
                "WkT": WkT,
                "WvT": WvT,
                "WoT": WoT,
                "bo": bo2,
                "masks": masks_by_half[half],
            }
        )

    if _compiled is None:
        _compiled = _build()
    res = run_bass_kernel_spmd(
        _compiled, in_maps, core_ids=list(range(NC)), trace=False
    )

    out = np.empty((B, S, H), np.float32)
    for c in range(NC):
        b, half = c // 2, c % 2
        out[b, half * SL : (half + 1) * SL] = res.results[c]["outT"].T
    return out
